# revision 6
# baseline (speedup 1.0000x reference)
"""Distributed Trainium2 (8 NeuronCores) kernel for a pre-LN transformer block.

Reference computation (B=2, T=2048, E=1024, H=16, D=64):
    h1 = LN(data); q,k,v = per-head projections; causal attention (scale E^-0.5);
    x = data + concat @ Wfc + bfc; out = x + relu(LN(x) @ W1 + b1) @ W2 + b2

Sharding (Ulysses-style, SPMD-uniform across the 8 cores):
  - rows (b,t) are sharded: core c owns rows [256c, 256c+256) of each batch
    (512 rows/core, held transposed as [E, 512], col order [b0|b1])
  - LN1 computed on local rows -> AllGather h1 (bf16, 1MB/rank)
  - heads sharded: core c owns heads {2c, 2c+1}; QKV + full-T causal
    attention for those heads (identical work on every core)
  - AllToAll attention output (bf16, 1MB/rank) back to row sharding
  - Wfc + residual + LN2 + FFN + residual computed on local rows
All matmuls run in bf16 (f32 PSUM accumulation); LN stats/softmax in f32.
"""
import numpy as np
import ml_dtypes

import concourse.bass as bass
import concourse.bacc as bacc
import concourse.tile as tile
from concourse import mybir
from concourse import bass_utils

FP32 = mybir.dt.float32
BF16 = mybir.dt.bfloat16
AF = mybir.ActivationFunctionType
OP = mybir.AluOpType

B, T, E, H, D = 2, 2048, 1024, 16, 64
NC = 8
RPB = T // NC            # 256 rows per batch per core
ROWS = B * RPB           # 512 local rows
NE = E // 128            # 8 tiles over E
F4 = 4 * E
NF = F4 // 128           # 32 tiles over 4E
NKT = T // 128           # 16 key tiles per batch
NCH = B * T // 512       # 8 row chunks of 512 over all gathered rows
EPS = 1e-5
SCALE = float(E) ** -0.5   # exactly 1/32
RG = [list(range(NC))]

_last_result = None  # BassKernelResults from the most recent run (for test harness)


def _layernorm(nc, tc, workp, statsp, eps1, x_tiles, g_col, b_col, out_factory,
               post, psname):
    """LayerNorm over the E (partition) axis of 8 [128, ROWS] f32 tiles.

    Column sums via PE ones-matmuls on internally-made bf16 casts; stats in
    f32; per-row scale/shift broadcast across partitions via K=1 matmuls.
    out_factory(e) -> AP (bf16 [128, ROWS]); post(e, ap) runs after the write.
    """
    ones128 = workp.tile([128, 1], BF16, name=f"{psname}_ones128",
                         tag="lno", bufs=2)
    nc.vector.memset(ones128[:], 1.0)
    ones1f = workp.tile([1, 128], FP32, name=f"{psname}_ones1f",
                        tag="lno1", bufs=2)
    nc.vector.memset(ones1f[:], 1.0)

    with tc.tile_pool(name=psname, bufs=1, space="PSUM") as ps:
        sum_ps = ps.tile([1, ROWS], FP32, name=f"{psname}_sum", tag="sum")
        ssq_ps = ps.tile([1, ROWS], FP32, name=f"{psname}_ssq", tag="ssq")
        for e in range(NE):
            xb = workp.tile([128, ROWS], BF16, name=f"{psname}_xb{e}",
                            tag="lnsrc", bufs=2)
            nc.vector.tensor_copy(xb[:], x_tiles[e][:])
            sq = workp.tile([128, ROWS], BF16, name=f"{psname}_sq{e}",
                            tag="lnsq", bufs=2)
            nc.scalar.activation(sq[:], x_tiles[e][:], AF.Square)
            nc.tensor.matmul(sum_ps[:], ones128[:], xb[:],
                             start=(e == 0), stop=(e == NE - 1))
            nc.tensor.matmul(ssq_ps[:], ones128[:], sq[:],
                             start=(e == 0), stop=(e == NE - 1))
        # stats, all [1, ROWS] f32
        mean = statsp.tile([1, ROWS], FP32, name=f"{psname}_mean", tag="v0")
        nc.vector.tensor_scalar_mul(mean[:], sum_ps[:], 1.0 / E)
        msq = statsp.tile([1, ROWS], FP32, name=f"{psname}_msq", tag="v1")
        nc.scalar.activation(msq[:], mean[:], AF.Square)
        ex2 = statsp.tile([1, ROWS], FP32, name=f"{psname}_ex2", tag="v2")
        nc.vector.tensor_scalar_mul(ex2[:], ssq_ps[:], 1.0 / E)
        var = statsp.tile([1, ROWS], FP32, name=f"{psname}_var", tag="v3")
        nc.vector.tensor_sub(var[:], ex2[:], msq[:])
        std = statsp.tile([1, ROWS], FP32, name=f"{psname}_std", tag="v4")
        nc.scalar.activation(std[:], var[:], AF.Sqrt, bias=eps1[:, 0:1])
        rstd = statsp.tile([1, ROWS], FP32, name=f"{psname}_rstd", tag="v5")
        nc.vector.reciprocal(rstd[:], std[:])
        nmr = statsp.tile([1, ROWS], FP32, name=f"{psname}_nmr", tag="v6")
        nc.vector.tensor_mul(nmr[:], mean[:], rstd[:])
        nmrn = statsp.tile([1, ROWS], FP32, name=f"{psname}_nmrn", tag="v7")
        nc.vector.tensor_scalar_mul(nmrn[:], nmr[:], -1.0)
        # broadcast across partitions via K=1 f32 matmuls
        bA = ps.tile([128, ROWS], FP32, name=f"{psname}_bA", tag="bA")
        nc.tensor.matmul(bA[:], ones1f[:], rstd[:], start=True, stop=True)
        bB = ps.tile([128, ROWS], FP32, name=f"{psname}_bB", tag="bB")
        nc.tensor.matmul(bB[:], ones1f[:], nmrn[:], start=True, stop=True)
        # normalize: out = (x * rstd - mean*rstd) * g + b
        for e in range(NE):
            t1 = workp.tile([128, ROWS], FP32, name=f"{psname}_t1_{e}",
                            tag="lnt1", bufs=2)
            nc.vector.tensor_mul(t1[:], x_tiles[e][:], bA[:])
            t2 = workp.tile([128, ROWS], FP32, name=f"{psname}_t2_{e}",
                            tag="lnt2", bufs=2)
            nc.vector.tensor_add(t2[:], t1[:], bB[:])
            o = out_factory(e)
            nc.vector.tensor_scalar(o[:], t2[:], g_col(e), b_col(e),
                                    OP.mult, OP.add)
            post(e, o)


def _build():
    nc = bacc.Bacc("TRN2", target_bir_lowering=False, debug=False, num_devices=NC)

    dataT_d = nc.dram_tensor("dataT", [E, ROWS], FP32, kind="ExternalInput")
    wq_d = nc.dram_tensor("wq", [E, 2 * D], BF16, kind="ExternalInput")
    wk_d = nc.dram_tensor("wk", [E, 2 * D], BF16, kind="ExternalInput")
    wv_d = nc.dram_tensor("wv", [E, 2 * D], BF16, kind="ExternalInput")
    wfc_d = nc.dram_tensor("wfc", [H * D, E], BF16, kind="ExternalInput")
    w1_d = nc.dram_tensor("w1", [E, F4], BF16, kind="ExternalInput")
    w2_d = nc.dram_tensor("w2", [F4, E], BF16, kind="ExternalInput")
    mask_d = nc.dram_tensor("mask", [128, 128], BF16, kind="ExternalInput")
    g1_d = nc.dram_tensor("g1", [E], FP32, kind="ExternalInput")
    be1_d = nc.dram_tensor("be1", [E], FP32, kind="ExternalInput")
    g2_d = nc.dram_tensor("g2", [E], FP32, kind="ExternalInput")
    be2_d = nc.dram_tensor("be2", [E], FP32, kind="ExternalInput")
    bfc_d = nc.dram_tensor("bfc", [E], FP32, kind="ExternalInput")
    b1_d = nc.dram_tensor("b1", [F4], FP32, kind="ExternalInput")
    b2_d = nc.dram_tensor("b2", [E], FP32, kind="ExternalInput")
    out_d = nc.dram_tensor("outT", [E, ROWS], FP32, kind="ExternalOutput")

    with tile.TileContext(nc) as tc:
        with (
            tc.tile_pool(name="constp", bufs=1) as constp,
            tc.tile_pool(name="datap", bufs=1) as datap,
            tc.tile_pool(name="workp", bufs=4) as workp,
            tc.tile_pool(name="statsp", bufs=1) as statsp,
            tc.tile_pool(name="xhp", bufs=1) as xhp,
            tc.tile_pool(name="dramp", bufs=1, space="DRAM") as dramp,
        ):
            # ---------- constant / input loads ----------
            mask_sb = constp.tile([128, 128], BF16, name="mask_sb", tag="mask")
            nc.sync.dma_start(out=mask_sb[:], in_=mask_d[:, :])
            ones1b = constp.tile([1, 128], BF16, name="ones1b", tag="ones1b")
            nc.vector.memset(ones1b[:], 1.0)
            eps1 = constp.tile([1, 1], FP32, name="eps1", tag="eps1")
            nc.vector.memset(eps1[:], EPS)
            vecs = {}
            for nm, dd, w in (("g1", g1_d, NE), ("be1", be1_d, NE), ("g2", g2_d, NE),
                              ("be2", be2_d, NE), ("bfc", bfc_d, NE), ("b2", b2_d, NE),
                              ("b1", b1_d, NF)):
                t = constp.tile([128, w], FP32, name=f"{nm}_sb", tag=nm)
                nc.sync.dma_start(out=t[:], in_=dd.ap().rearrange("(a b) -> b a", b=128))
                vecs[nm] = t

            data_t = []
            for e in range(NE):
                dt_ = datap.tile([128, ROWS], FP32, name=f"data{e}", tag=f"data{e}")
                nc.sync.dma_start(out=dt_[:], in_=dataT_d[128 * e:128 * (e + 1), :])
                data_t.append(dt_)

            # DRAM bounce buffers for the collectives
            ag_in = dramp.tile([E, ROWS], BF16, name="ag_in", tag="ag_in")
            ag_out = dramp.tile([NC * E, ROWS], BF16, name="ag_out", tag="ag_out",
                                addr_space="Shared")
            a2a_in = dramp.tile([NC * 128, ROWS], BF16, name="a2a_in", tag="a2a_in")
            a2a_out = dramp.tile([NC * 128, ROWS], BF16, name="a2a_out",
                                 tag="a2a_out")

            # ---------- phase 1: LN1 on local rows ----------
            def h1_factory(e):
                return workp.tile([128, ROWS], BF16, name=f"h1_{e}", tag="h1", bufs=3)

            def h1_post(e, ap):
                nc.sync.dma_start(out=ag_in[128 * e:128 * (e + 1), :], in_=ap[:])

            _layernorm(nc, tc, workp, statsp, eps1, data_t,
                       lambda e: vecs["g1"][:, e:e + 1],
                       lambda e: vecs["be1"][:, e:e + 1],
                       h1_factory, h1_post, "ln1")

            # ---------- phase 2: AllGather h1 ----------
            nc.gpsimd.collective_compute(
                "AllGather", OP.bypass, replica_groups=RG,
                ins=[ag_in[:, :].opt()], outs=[ag_out[:, :].opt()])

            with tc.tile_pool(name="wfcp", bufs=1) as wfcp:
                with (
                    tc.tile_pool(name="qtp", bufs=1) as qtp,
                    tc.tile_pool(name="vp", bufs=1) as vp,
                    tc.tile_pool(name="clp", bufs=1) as clp,
                ):
                    QT = qtp.tile([128, B * T], BF16, name="QT", tag="QT")
                    KT = qtp.tile([128, B * T], BF16, name="KT", tag="KT")
                    v_t = []
                    for rt in range(B * NKT):
                        vt = vp.tile([128, 130], BF16, name=f"v{rt}", tag=f"v{rt}")
                        # ones columns at 64 and 129 (softmax denominator trick)
                        nc.vector.memset(vt[:, 64:65], 1.0)
                        nc.vector.memset(vt[:, 129:130], 1.0)
                        v_t.append(vt)
                    concatL = clp.tile([128, B * T], BF16, name="concatL",
                                       tag="concatL")

                    # ------- phase 3: QKV for the 2 local heads, chunked -------
                    # global row chunk ch (512 rows) = batch ch//4, ranks
                    # {2*(ch%4), 2*(ch%4)+1} x 256 rows each in ag_out
                    with (
                        tc.tile_pool(name="h1cp", bufs=1) as h1cp,
                        tc.tile_pool(name="wqkvp", bufs=1) as wqkvp,
                        tc.tile_pool(name="psqkv", bufs=2, space="PSUM") as psqkv,
                    ):
                        wq_t, wk_t, wv_t = [], [], []
                        for nm, dd, lst in (("wq", wq_d, wq_t), ("wk", wk_d, wk_t),
                                            ("wv", wv_d, wv_t)):
                            for e in range(NE):
                                t = wqkvp.tile([128, 2 * D], BF16,
                                               name=f"{nm}t{e}", tag=f"{nm}{e}")
                                nc.sync.dma_start(
                                    out=t[:], in_=dd[128 * e:128 * (e + 1), :])
                                lst.append(t)
                        for ch in range(NCH):
                            b, i4 = divmod(ch, 4)
                            h1c = []
                            for e in range(NE):
                                hc = h1cp.tile([128, 512], BF16,
                                               name=f"h1c{ch}_{e}", tag=f"h1c{e}",
                                               bufs=2)
                                for rr in range(2):
                                    r = 2 * i4 + rr
                                    src = ag_out[r * E + 128 * e:
                                                 r * E + 128 * (e + 1),
                                                 b * RPB:(b + 1) * RPB]
                                    nc.sync.dma_start(
                                        out=hc[:, RPB * rr:RPB * (rr + 1)], in_=src)
                                h1c.append(hc)
                            for dst, wt in ((QT, wq_t), (KT, wk_t)):
                                ps = psqkv.tile([128, 512], FP32,
                                                name=f"psq{ch}", tag="mm")
                                for e in range(NE):
                                    nc.tensor.matmul(
                                        ps[:], wt[e][:], h1c[e][:],
                                        start=(e == 0), stop=(e == NE - 1))
                                nc.vector.tensor_copy(
                                    dst[:, 512 * ch:512 * (ch + 1)], ps[:])
                            for rti in range(4):
                                rt = 4 * ch + rti
                                ps = psqkv.tile([128, 128], FP32,
                                                name=f"psv{rt}", tag="mm")
                                for e in range(NE):
                                    nc.tensor.matmul(
                                        ps[:],
                                        h1c[e][:, 128 * rti:128 * (rti + 1)],
                                        wv_t[e][:],
                                        start=(e == 0), stop=(e == NE - 1))
                                nc.vector.tensor_copy(v_t[rt][:, 0:64], ps[:, 0:64])
                                nc.vector.tensor_copy(v_t[rt][:, 65:129],
                                                      ps[:, 64:128])

                    # prefetch Wfc while attention runs
                    wfc_t = []
                    for s in range(NE):
                        t = wfcp.tile([128, E], BF16, name=f"wfct{s}", tag=f"wfc{s}")
                        nc.sync.dma_start(out=t[:], in_=wfc_d[128 * s:128 * (s + 1), :])
                        wfc_t.append(t)

                    # ------- phase 4: causal attention for 2 heads -------
                    with (
                        tc.tile_pool(name="pst", bufs=4, space="PSUM") as pst,
                        tc.tile_pool(name="pot", bufs=2, space="PSUM") as pot,
                        tc.tile_pool(name="prb", bufs=1, space="PSUM") as prb,
                    ):
                        for b in range(B):
                            for qc in range(T // 512):
                                q0 = 512 * qc
                                nk = 4 * qc + 4
                                ots = []
                                for hi in range(2):
                                    ots.append(pot.tile([65, 512], FP32,
                                                        name=f"ot{b}_{qc}_{hi}",
                                                        tag="ot"))
                                for k in range(nk):
                                    off = max(0, 128 * k - q0)
                                    rt = b * NKT + k
                                    for hi in range(2):
                                        hp = slice(64 * hi, 64 * (hi + 1))
                                        st = pst.tile([128, 512], FP32,
                                                      name=f"st{b}_{qc}_{k}_{hi}",
                                                      tag="st")
                                        nc.tensor.matmul(
                                            st[:, off:512],
                                            KT[hp, b * T + 128 * k:
                                               b * T + 128 * (k + 1)],
                                            QT[hp, b * T + q0 + off:
                                               b * T + q0 + 512],
                                            start=True, stop=True,
                                            tile_position=(64 * hi, 0))
                                        pexp = workp.tile(
                                            [128, 512], BF16,
                                            name=f"pex{b}_{qc}_{k}_{hi}",
                                            tag="pexp", bufs=4)
                                        nc.scalar.activation(pexp[:, off:512],
                                                             st[:, off:512],
                                                             AF.Exp, scale=SCALE)
                                        if k >= 4 * qc:  # diagonal: causal mask
                                            nc.vector.tensor_mul(
                                                pexp[:, off:off + 128],
                                                pexp[:, off:off + 128], mask_sb[:])
                                        nc.tensor.matmul(
                                            ots[hi][:, off:512],
                                            v_t[rt][:, 65 * hi:65 * hi + 65],
                                            pexp[:, off:512],
                                            start=(k == 0), stop=(k == nk - 1))
                                for hi in range(2):
                                    rc = statsp.tile([1, 512], FP32,
                                                     name=f"rc{b}_{qc}_{hi}",
                                                     tag="rc")
                                    nc.vector.reciprocal(rc[:], ots[hi][64:65, :])
                                    rcb = statsp.tile([1, 512], BF16,
                                                      name=f"rcb{b}_{qc}_{hi}",
                                                      tag="rcb")
                                    nc.vector.tensor_copy(rcb[:], rc[:])
                                    rb = prb.tile([64, 512], FP32,
                                                  name=f"rb{b}_{qc}_{hi}", tag="rb")
                                    nc.tensor.matmul(rb[:], ones1b[:, 0:64], rcb[:],
                                                     start=True, stop=True)
                                    rbs = workp.tile([64, 512], FP32,
                                                     name=f"rbs{b}_{qc}_{hi}",
                                                     tag="rbs", bufs=2)
                                    nc.scalar.copy(rbs[:], rb[:])
                                    nc.vector.tensor_mul(
                                        concatL[64 * hi:64 * (hi + 1),
                                                b * T + q0: b * T + q0 + 512],
                                        ots[hi][0:64, :], rbs[:])

                    # ------- phase 5: AllToAll back to row sharding -------
                    for j in range(NC):
                        nc.sync.dma_start(out=a2a_in[128 * j:128 * (j + 1), 0:RPB],
                                          in_=concatL[:, RPB * j:RPB * (j + 1)])
                        nc.sync.dma_start(
                            out=a2a_in[128 * j:128 * (j + 1), RPB:ROWS],
                            in_=concatL[:, T + RPB * j:T + RPB * (j + 1)])
                nc.gpsimd.collective_compute(
                    "AllToAll", OP.bypass, replica_groups=RG,
                    ins=[a2a_in[:, :].opt()], outs=[a2a_out[:, :].opt()])

                # ---------- phase 6: Wfc + residual -> xT ----------
                x_t, h2_t = [], []
                for e in range(NE):
                    x_t.append(xhp.tile([128, ROWS], FP32, name=f"x{e}",
                                        tag=f"x{e}"))
                    h2_t.append(xhp.tile([128, ROWS], BF16, name=f"h2_{e}",
                                         tag=f"h2{e}"))
                with (
                    tc.tile_pool(name="ccp", bufs=1) as ccp,
                    tc.tile_pool(name="psfc", bufs=2, space="PSUM") as psfc,
                ):
                    cc_t = []
                    for s in range(NC):
                        t = ccp.tile([128, ROWS], BF16, name=f"cc{s}", tag=f"cc{s}")
                        nc.sync.dma_start(out=t[:],
                                          in_=a2a_out[128 * s:128 * (s + 1), :])
                        cc_t.append(t)
                    for e in range(NE):
                        ps = psfc.tile([128, ROWS], FP32, name=f"psx{e}", tag="mm")
                        for s in range(NC):
                            nc.tensor.matmul(ps[:],
                                             wfc_t[s][:, 128 * e:128 * (e + 1)],
                                             cc_t[s][:],
                                             start=(s == 0), stop=(s == NC - 1))
                        nc.vector.scalar_tensor_tensor(
                            x_t[e][:], ps[:], vecs["bfc"][:, e:e + 1], data_t[e][:],
                            OP.add, OP.add)

            # ---------- phase 7: LN2 ----------
            _layernorm(nc, tc, workp, statsp, eps1, x_t,
                       lambda e: vecs["g2"][:, e:e + 1],
                       lambda e: vecs["be2"][:, e:e + 1],
                       lambda e: h2_t[e], lambda e, ap: None, "ln2")

            # ---------- phase 8: FFN ----------
            # w1 streamed per f-tile: w1f[p, 128e+c] = w1[128e+p, 128f+c]
            with (
                tc.tile_pool(name="w1p", bufs=1) as w1p,
                tc.tile_pool(name="rtp", bufs=1) as rtp,
            ):
                r_t = []
                for f in range(NF):
                    r_t.append(rtp.tile([128, ROWS], BF16, name=f"r{f}",
                                        tag=f"r{f}"))
                with tc.tile_pool(name="psz", bufs=2, space="PSUM") as psz:
                    for f in range(NF):
                        w1f = w1p.tile([128, E], BF16, name=f"w1f{f}", tag="w1f",
                                       bufs=4)
                        src = w1_d[:, 128 * f:128 * (f + 1)].rearrange(
                            "(a p) c -> p a c", p=128)
                        nc.sync.dma_start(
                            out=w1f[:].rearrange("p (a c) -> p a c", c=128),
                            in_=src)
                        ps = psz.tile([128, ROWS], FP32, name=f"psz{f}", tag="mm")
                        for e in range(NE):
                            nc.tensor.matmul(ps[:],
                                             w1f[:, 128 * e:128 * (e + 1)],
                                             h2_t[e][:],
                                             start=(e == 0), stop=(e == NE - 1))
                        nc.scalar.activation(r_t[f][:], ps[:], AF.Relu,
                                             bias=vecs["b1"][:, f:f + 1])
                with (
                    tc.tile_pool(name="w2p", bufs=3) as w2p,
                    tc.tile_pool(name="psff", bufs=1, space="PSUM") as psff,
                ):
                    ff_ps = []
                    for e in range(NE):
                        ff_ps.append(psff.tile([128, ROWS], FP32,
                                               name=f"ff{e}", tag=f"ff{e}"))
                    for f in range(NF):
                        w2t = w2p.tile([128, E], BF16, name=f"w2t{f}", tag="w2")
                        nc.sync.dma_start(out=w2t[:],
                                          in_=w2_d[128 * f:128 * (f + 1), :])
                        for e in range(NE):
                            nc.tensor.matmul(ff_ps[e][:],
                                             w2t[:, 128 * e:128 * (e + 1)],
                                             r_t[f][:],
                                             start=(f == 0), stop=(f == NF - 1))
                    for e in range(NE):
                        o = workp.tile([128, ROWS], FP32, name=f"o{e}",
                                       tag="o", bufs=2)
                        nc.vector.scalar_tensor_tensor(
                            o[:], ff_ps[e][:], vecs["b2"][:, e:e + 1], x_t[e][:],
                            OP.add, OP.add)
                        nc.sync.dma_start(out=out_d[128 * e:128 * (e + 1), :],
                                          in_=o[:])

    nc.compile()
    return nc


def _shard(inputs):
    bf = ml_dtypes.bfloat16
    data = np.asarray(inputs["data"], np.float32)
    Wq = np.asarray(inputs["Wq"], np.float32)
    Wk = np.asarray(inputs["Wk"], np.float32)
    Wv = np.asarray(inputs["Wv"], np.float32)
    wfc = np.ascontiguousarray(np.asarray(inputs["Wfc"], np.float32).astype(bf))
    w1 = np.ascontiguousarray(np.asarray(inputs["W1"], np.float32).astype(bf))
    w2 = np.ascontiguousarray(np.asarray(inputs["W2"], np.float32).astype(bf))
    kk, qq = np.meshgrid(np.arange(128), np.arange(128), indexing="ij")
    mask = np.ascontiguousarray((kk <= qq).astype(bf))
    common = dict(wfc=wfc, w1=w1, w2=w2, mask=mask)
    for nm in ("g1", "be1", "g2", "be2", "bfc", "b1", "b2"):
        common[nm] = np.ascontiguousarray(np.asarray(inputs[nm], np.float32))
    in_maps = []
    for c in range(NC):
        rows = np.concatenate([data[0, RPB * c:RPB * (c + 1)],
                               data[1, RPB * c:RPB * (c + 1)]], axis=0)  # [512, E]
        m = dict(common)
        m["dataT"] = np.ascontiguousarray(rows.T)
        m["wq"] = np.ascontiguousarray(
            np.concatenate([Wq[2 * c], Wq[2 * c + 1]], axis=1).astype(bf))
        m["wk"] = np.ascontiguousarray(
            np.concatenate([Wk[2 * c], Wk[2 * c + 1]], axis=1).astype(bf))
        m["wv"] = np.ascontiguousarray(
            np.concatenate([Wv[2 * c], Wv[2 * c + 1]], axis=1).astype(bf))
        in_maps.append(m)
    return in_maps


_nc_cache = None


def kernel(**inputs):
    global _last_result, _nc_cache
    if _nc_cache is None:
        _nc_cache = _build()
    in_maps = _shard(inputs)
    res = bass_utils.run_bass_kernel_spmd(
        _nc_cache, in_maps, core_ids=list(range(NC)))
    _last_result = res
    out = np.zeros((B, T, E), np.float32)
    for c in range(NC):
        ot = np.asarray(res.results[c]["outT"], np.float32)  # [E, 512]
        out[0, RPB * c:RPB * (c + 1)] = ot[:, 0:RPB].T
        out[1, RPB * c:RPB * (c + 1)] = ot[:, RPB:ROWS].T
    return out
